# revision 1
# baseline (speedup 1.0000x reference)
"""Causal self-attention (softmax over the QUERY axis) for Trainium2, 8 cores.

Reference semantics (note the quirk -- softmax over dim=1, the query axis):
    q = x @ Wq.T ; k = x @ Wk.T ; v = x @ Wv.T          (per batch)
    s[q_, k_] = <q[q_], k[k_]>,  masked -inf where k_ > q_
    attn = softmax(s / sqrt(D), axis=q_)                 (normalize per key column)
    out[q_, :] = sum_k attn[q_, k_] * v[k_, :]

Because the softmax normalizes each key COLUMN over queries, the whole thing
factors as  out = W @ (v / Z)  with
    W[k_, q_] = exp(s^T * scale) * causal_mask,   Z[k_] = sum_q W[k_, q_].

Algebraic folding: s[q_, k_] = x[q_] . A . x[k_]  with A = Wq^T @ Wk, so with
y = x_k @ A^T the scores come straight from x (no q/k projections needed):
    s^T[k_, q_] = sum_d y[k_, d] * x[q_, d].
A is computed once on the host.

Sharding: 8 cores = 4 batches x 2 key-shards.  Key columns are interleaved by
parity (core h in {0,1} owns original key positions 2*m + h) so the causal
work balances AND every core runs the identical program (pure SPMD); only the
input data differs per core.  Each core computes a partial output (sum over
its own keys); the host adds the two partials per batch.

Device layout (per core, b = batch, h = parity):
    xT   [D, N]  bf16  x[b].T
    a2   [D, D]  bf16  A^T = Wk^T @ Wq   (layout [e, d])
    wvT  [D, D]  bf16  Wv.T              (layout [e, o])
    maskbias [128, 256] f32  0 where valid, -1e9 where masked (depends on h)
    out  [N, D]  f32   partial output

All matmul inputs are bf16 (PE full rate), accumulation fp32 in PSUM.
"""
import numpy as np
import ml_dtypes
from contextlib import ExitStack

import concourse.bass as bass
import concourse.tile as tile
import concourse.bacc as bacc
import concourse.mybir as mybir
from concourse.bass_utils import run_bass_kernel_spmd

B, N, D = 4, 2048, 1024
NT = N // 128          # 16 query tiles
ET = D // 128          # 8 contraction tiles
G = 8                  # key groups per core (128 interleaved keys each)
SCALE = 1.0 / np.sqrt(D).astype(np.float32)
NEGBIG = -1.0e9

BF = mybir.dt.bfloat16
F32 = mybir.dt.float32

# packed offsets of each group's score row-block inside the wT buffer
WOFF = []
_o = 0
for _g in range(G):
    WOFF.append(_o)
    _o += N - 256 * _g
WTOT = _o  # 9216


def _score_chunks(g):
    """(q0, width) chunks covering the valid span [256g, N) of group g.
    The first chunk always contains the 256 masked columns; widths 512/256."""
    width = N - 256 * g
    q0 = 256 * g
    chunks = []
    if (width // 256) % 2 == 1:
        chunks.append((q0, 256))
        q0 += 256
    while q0 < N:
        chunks.append((q0, 512))
        q0 += 512
    return chunks


def _emit_body(nc, tc, ctx, pools, aps, stages="full"):
    (xpool, wpool, ypool, vpool, vppool, zpool, stpool, ps, avps) = pools
    (xT_d, xkT_d, a2_d, wvT_d, mb_sb, wt_sb, out_d) = aps

    # ---- loads: what y-proj needs first (xk + a2), then wvT, then xT ----
    xk = []
    for t in range(ET):
        xktile = xpool.tile([128, D], BF, tag="xk")
        nc.sync.dma_start(xktile[:], xkT_d[t * 128:(t + 1) * 128, :])
        xk.append(xktile)
    a2t = []
    for t in range(ET):
        w = wpool.tile([128, D], BF, tag="w")
        nc.sync.dma_start(w[:], a2_d[t * 128:(t + 1) * 128, :])
        a2t.append(w)
    wvt = []
    for t in range(ET):
        w = wpool.tile([128, D], BF, tag="w")
        nc.sync.dma_start(w[:], wvT_d[t * 128:(t + 1) * 128, :])
        wvt.append(w)
    xt = []
    for t in range(ET):
        xtile = xpool.tile([128, N], BF, tag="xt")
        nc.sync.dma_start(xtile[:], xT_d[t * 128:(t + 1) * 128, :])
        xt.append(xtile)

    # ---- y projection: yT[d_tile][:, m] = sum_e a2[e, d] * xkT[e, m] ----
    yt = []
    for dt_ in range(ET):
        ytile = ypool.tile([128, D], BF, tag="yt")   # owned m = 1024 cols
        for c in range(2):                            # m chunks of 512
            pt = ps.tile([128, 512], F32, tag="ps")
            for e in range(ET):
                nc.tensor.matmul(
                    pt[:],
                    a2t[e][:, dt_ * 128:(dt_ + 1) * 128],
                    xk[e][:, c * 512:(c + 1) * 512],
                    start=(e == 0), stop=(e == ET - 1),
                )
            if stages == "full":
                nc.vector.tensor_copy(ytile[:, c * 512:(c + 1) * 512], pt[:])
        yt.append(ytile)

    # ---- v projection: v[g][i, o] = sum_e xkT[e, 128g+i] * wvT[e, o] ----
    vt = []
    for g in range(G):
        vtile = vpool.tile([128, D], F32, tag="v")
        for c in range(2):                            # o chunks of 512
            pt = ps.tile([128, 512], F32, tag="ps")
            for e in range(ET):
                nc.tensor.matmul(
                    pt[:],
                    xk[e][:, g * 128:(g + 1) * 128],
                    wvt[e][:, c * 512:(c + 1) * 512],
                    start=(e == 0), stop=(e == ET - 1),
                )
            if stages == "full":
                nc.vector.tensor_copy(vtile[:, c * 512:(c + 1) * 512], pt[:])
        vt.append(vtile)

    # ---- per group: scores^T -> exp/mask/Z -> v' ; then AV for its q-tiles ----
    vp = [None] * G
    for g in range(G):
        chunks = _score_chunks(g)
        nch = len(chunks)
        if stages == "full":
            zp = zpool.tile([128, nch], F32, tag="zp")
        else:
            zp = None
        for ci, (q0, w) in enumerate(chunks):
            pt = ps.tile([128, 512], F32, tag="ps")
            for dt_ in range(ET):
                lhs = (yt[dt_][:, g * 128:(g + 1) * 128] if stages == "full"
                       else xk[dt_][:, g * 128:(g + 1) * 128])
                nc.tensor.matmul(
                    pt[:, :w],
                    lhs,
                    xt[dt_][:, q0:q0 + w],
                    start=(dt_ == 0), stop=(dt_ == ET - 1),
                )
            if stages == "full":
                if ci == 0:
                    # masked (diagonal) region = first 256 valid columns
                    nc.vector.tensor_add(pt[:, :256], pt[:, :256], mb_sb[:])
                nc.scalar.activation(
                    wt_sb[:, WOFF[g] + (q0 - 256 * g): WOFF[g] + (q0 - 256 * g) + w],
                    pt[:, :w],
                    mybir.ActivationFunctionType.Exp,
                    scale=float(SCALE),
                    accum_out=zp[:, ci:ci + 1],
                )
        if stages == "full":
            vptile = vppool.tile([128, D], BF, tag="vp")
        else:
            vptile = None
        if stages == "full":
            z = zpool.tile([128, 1], F32, tag="z")
            nc.vector.tensor_reduce(z[:], zp[:], axis=mybir.AxisListType.X,
                                    op=mybir.AluOpType.add)
            rz = zpool.tile([128, 1], F32, tag="rz")
            nc.vector.reciprocal(rz[:], z[:])
            nc.vector.tensor_scalar_mul(vptile[:], vt[g][:], rz[:])
        vp[g] = vptile

        # AV for q-tiles 2g and 2g+1 (they need groups 0..g only)
        for qt in (2 * g, 2 * g + 1):
            stage = stpool.tile([128, D], F32, tag="st")
            for oc in range(2):
                apt = avps.tile([128, 512], F32, tag="av")
                for gg in range(g + 1):
                    if stages == "full":
                        lhs = wt_sb[:, WOFF[gg] + 128 * qt - 256 * gg:
                                       WOFF[gg] + 128 * qt - 256 * gg + 128]
                        rhs = vp[gg][:, oc * 512:(oc + 1) * 512]
                    else:
                        lhs = xk[gg][:, :128]
                        rhs = xk[gg][:, oc * 512:(oc + 1) * 512]
                    nc.tensor.matmul(apt[:], lhs, rhs,
                                     start=(gg == 0), stop=(gg == g))
                if stages == "full" or qt == NT - 1:
                    nc.vector.tensor_copy(stage[:, oc * 512:(oc + 1) * 512], apt[:])
            if stages == "full" or qt == NT - 1:
                nc.sync.dma_start(out_d[qt * 128:(qt + 1) * 128, :], stage[:])


def build_program(with_loop=False, max_iters=64, stages="full"):
    """Build and compile the SPMD program. Returns the compiled Bacc."""
    nc = bacc.Bacc("TRN2", target_bir_lowering=False, debug=False, num_devices=8)
    xT_d = nc.dram_tensor("xT", [D, N], BF, kind="ExternalInput").ap()
    xkT_d = nc.dram_tensor("xkT", [D, D], BF, kind="ExternalInput").ap()
    a2_d = nc.dram_tensor("a2", [D, D], BF, kind="ExternalInput").ap()
    wvT_d = nc.dram_tensor("wvT", [D, D], BF, kind="ExternalInput").ap()
    mb_d = nc.dram_tensor("maskbias", [128, 256], F32, kind="ExternalInput").ap()
    out_d = nc.dram_tensor("out", [N, D], F32, kind="ExternalOutput").ap()
    if with_loop:
        n_d = nc.dram_tensor("niter", [1, 1], mybir.dt.int32,
                             kind="ExternalInput").ap()

    with tile.TileContext(nc) as tc:
        with ExitStack() as ctx:
            persist = ctx.enter_context(tc.tile_pool(name="persist", bufs=1))
            xpool = ctx.enter_context(tc.tile_pool(name="xT", bufs=ET))
            wpool = ctx.enter_context(tc.tile_pool(name="weights", bufs=2 * ET))
            ypool = ctx.enter_context(tc.tile_pool(name="yT", bufs=ET))
            vpool = ctx.enter_context(tc.tile_pool(name="v", bufs=G))
            vppool = ctx.enter_context(tc.tile_pool(name="vp", bufs=G))
            zpool = ctx.enter_context(tc.tile_pool(name="z", bufs=3 * G))
            stpool = ctx.enter_context(tc.tile_pool(name="stage", bufs=4))
            ps = ctx.enter_context(tc.tile_pool(name="ps", bufs=4, space="PSUM"))
            avps = ctx.enter_context(tc.tile_pool(name="avps", bufs=4, space="PSUM"))

            mb_sb = persist.tile([128, 256], F32, tag="mb")
            nc.sync.dma_start(mb_sb[:], mb_d[:])
            wt_sb = persist.tile([128, WTOT], BF, tag="wt")

            pools = (xpool, wpool, ypool, vpool, vppool, zpool, stpool, ps, avps)
            aps = (xT_d, xkT_d, a2_d, wvT_d, mb_sb, wt_sb, out_d)

            if with_loop:
                n_sb = persist.tile([1, 1], mybir.dt.int32, tag="niter")
                nc.sync.dma_start(n_sb[:], n_d[:])
                regs = []
                with tc.tile_critical():
                    for e, eng in nc.engines.items():
                        r = eng.alloc_register(f"niter_{e.name}")
                        eng.reg_load(r, n_sb[0:1, 0:1])
                        regs.append(r)
                n_val = nc.snap(bass.RegisterHandles(regs), min_val=0,
                                max_val=max_iters)
                with tc.For_i(0, n_val, 1):
                    _emit_body(nc, tc, ctx, pools, aps, stages)
            else:
                _emit_body(nc, tc, ctx, pools, aps, stages)

    nc.compile()
    return nc


def prepare_in_maps(x, Wq, Wk, Wv, niter=None):
    """Host-side sharding: per-core input maps (8 cores)."""
    x = np.asarray(x, dtype=np.float32)
    A2 = (np.asarray(Wk, np.float32).T @ np.asarray(Wq, np.float32))  # [e, d]
    a2_bf = A2.astype(ml_dtypes.bfloat16)
    wvT_bf = np.asarray(Wv, np.float32).T.astype(ml_dtypes.bfloat16)  # [e, o]
    mb = []
    for h in range(2):
        i = np.arange(128)[:, None]
        j = np.arange(256)[None, :]
        mb.append(np.where(j >= 2 * i + h, 0.0, NEGBIG).astype(np.float32))
    in_maps = []
    for c in range(8):
        b, h = c // 2, c % 2
        xTb = x[b].T.astype(ml_dtypes.bfloat16)
        m = {
            "xT": xTb,
            "xkT": np.ascontiguousarray(xTb[:, h::2]),
            "a2": a2_bf,
            "wvT": wvT_bf,
            "maskbias": mb[h],
        }
        if niter is not None:
            m["niter"] = np.array([[niter]], dtype=np.int32)
        in_maps.append(m)
    return in_maps


_CACHE = {}


def kernel(x, Wq, Wk, Wv):
    if "nc" not in _CACHE:
        _CACHE["nc"] = build_program(with_loop=False)
    nc = _CACHE["nc"]
    in_maps = prepare_in_maps(x, Wq, Wk, Wv)
    res = run_bass_kernel_spmd(nc, in_maps, list(range(8)), trace=False)
    out = np.empty((B, N, D), np.float32)
    for b in range(B):
        out[b] = res.results[2 * b]["out"] + res.results[2 * b + 1]["out"]
    return out



# revision 2
# speedup vs baseline: 1.9036x; 1.9036x over previous
"""Causal self-attention (softmax over the QUERY axis) for Trainium2, 8 cores.

Reference semantics (note the quirk -- softmax over dim=1, the query axis):
    q = x @ Wq.T ; k = x @ Wk.T ; v = x @ Wv.T          (per batch)
    s[q_, k_] = <q[q_], k[k_]>,  masked -inf where k_ > q_
    attn = softmax(s / sqrt(D), axis=q_)                 (normalize per key column)
    out[q_, :] = sum_k attn[q_, k_] * v[k_, :]

Because the softmax normalizes each key COLUMN over queries, the whole thing
factors as  out = W @ (v / Z)  with
    W[k_, q_] = exp(s^T * scale) * causal_mask,   Z[k_] = sum_q W[k_, q_].

Algebraic folding: s[q_, k_] = x[q_] . A . x[k_]  with A = Wq^T @ Wk, so with
y = x_k @ A^T the scores come straight from x (no q/k projections needed):
    s^T[k_, q_] = sum_d y[k_, d] * x[q_, d].
A is computed once on the host.

Sharding: 8 cores = 4 batches x 2 key-shards.  Key columns are interleaved by
parity (core h in {0,1} owns original key positions 2*m + h) so the causal
work balances AND every core runs the identical program (pure SPMD); only the
input data differs per core.  Each core computes a partial output (sum over
its own keys, stored bf16); the host adds the two partials per batch in f32.

Device layout (per core, b = batch, h = parity):
    xT   [D, N]  bf16  x[b].T
    a2   [D, D]  bf16  A^T = Wk^T @ Wq   (layout [e, d])
    wvT  [D, D]  bf16  Wv.T              (layout [e, o])
    maskbias [128, 256] f32  0 where valid, -1e9 where masked (depends on h)
    out  [N, D]  bf16  partial output

All matmul inputs are bf16 (PE full rate), accumulation fp32 in PSUM.

Perf notes (measured on HW):
  - PE streams 1 rhs column/cycle at an effective ~1.93 GHz with all 8 cores
    busy; total 278528 streamed columns/core -> ~144 us PE floor.
  - LDWEIGHTS is fully hidden (reuse=1 vs 8 measured identical).
  - Input DMAs are split across the two HW DGE queues (SP + Activation) and
    interleaved so the y-projection starts ~2 us in, not ~10 us.
  - Optional warmup matmuls heat the PE clock gate during the DMA ramp.
"""
import numpy as np
import ml_dtypes
from contextlib import ExitStack

import concourse.bass as bass
import concourse.tile as tile
import concourse.bacc as bacc
import concourse.mybir as mybir
from concourse.bass_utils import run_bass_kernel_spmd

B, N, D = 4, 2048, 1024
NT = N // 128          # 16 query tiles
ET = D // 128          # 8 contraction tiles
G = 8                  # key groups per core (128 interleaved keys each)
SCALE = 1.0 / np.sqrt(D).astype(np.float32)
NEGBIG = -1.0e9

BF = mybir.dt.bfloat16
F32 = mybir.dt.float32

# packed offsets of each group's score row-block inside the wT buffer
WOFF = []
_o = 0
for _g in range(G):
    WOFF.append(_o)
    _o += N - 256 * _g
WTOT = _o  # 9216


def _score_chunks(g):
    """(q0, width) chunks covering the valid span [256g, N) of group g.
    The first chunk always contains the 256 masked columns; widths 512/256."""
    width = N - 256 * g
    q0 = 256 * g
    chunks = []
    if (width // 256) % 2 == 1:
        chunks.append((q0, 256))
        q0 += 256
    while q0 < N:
        chunks.append((q0, 512))
        q0 += 512
    return chunks


def _emit_body(nc, tc, ctx, pools, aps, stages="full", warmup=8):
    (xpool, wpool, ypool, vpool, vppool, zpool, stpool, ps, avps) = pools
    (xT_d, xkT_d, a2_d, wvT_d, mb_sb, wt_sb, wz_sb, out_d) = aps

    # ---- PE warmup: standalone matmuls on a zeroed tile; no data deps so
    # they run first and heat the clock gate while input DMAs stream in.
    for i in range(warmup):
        wp = ps.tile([128, 512], F32, tag="ps")
        nc.tensor.matmul(wp[:], wz_sb[:, :128], wz_sb[:], start=True, stop=True)

    # ---- loads: interleave the y-proj operands (a2 on the Activation HW
    # queue, xk on the SP queue) so the first chain can start ~2 us in; the
    # later stages' tensors (wvT on Act, xT on SP) stream behind them.
    xk = []
    a2t = []
    for t in range(ET):
        xktile = xpool.tile([128, D], BF, tag="xk")
        nc.sync.dma_start(xktile[:], xkT_d[t * 128:(t + 1) * 128, :])
        xk.append(xktile)
        w = wpool.tile([128, D], BF, tag="w")
        nc.scalar.dma_start(w[:], a2_d[t * 128:(t + 1) * 128, :])
        a2t.append(w)
    wvt = []
    for t in range(ET):
        w = wpool.tile([128, D], BF, tag="w")
        nc.scalar.dma_start(w[:], wvT_d[t * 128:(t + 1) * 128, :])
        wvt.append(w)
    xt = []
    for t in range(ET):
        xtile = xpool.tile([128, N], BF, tag="xt")
        nc.sync.dma_start(xtile[:], xT_d[t * 128:(t + 1) * 128, :])
        xt.append(xtile)

    # ---- y projection: yT[d_tile][:, m] = sum_e a2[e, d] * xkT[e, m] ----
    yt = []
    for dt_ in range(ET):
        ytile = ypool.tile([128, D], BF, tag="yt")   # owned m = 1024 cols
        for c in range(2):                            # m chunks of 512
            pt = ps.tile([128, 512], F32, tag="ps")
            for e in range(ET):
                nc.tensor.matmul(
                    pt[:],
                    a2t[e][:, dt_ * 128:(dt_ + 1) * 128],
                    xk[e][:, c * 512:(c + 1) * 512],
                    start=(e == 0), stop=(e == ET - 1),
                )
            if stages == "full":
                nc.vector.tensor_copy(ytile[:, c * 512:(c + 1) * 512], pt[:])
        yt.append(ytile)

    # ---- v projection: v[g][i, o] = sum_e xkT[e, 128g+i] * wvT[e, o] ----
    vt = []
    for g in range(G):
        vtile = vpool.tile([128, D], F32, tag="v")
        for c in range(2):                            # o chunks of 512
            pt = ps.tile([128, 512], F32, tag="ps")
            for e in range(ET):
                nc.tensor.matmul(
                    pt[:],
                    xk[e][:, g * 128:(g + 1) * 128],
                    wvt[e][:, c * 512:(c + 1) * 512],
                    start=(e == 0), stop=(e == ET - 1),
                )
            if stages == "full":
                nc.vector.tensor_copy(vtile[:, c * 512:(c + 1) * 512], pt[:])
        vt.append(vtile)

    # ---- per group: scores^T -> exp/mask/Z -> v' ; then AV for its q-tiles ----
    vp = [None] * G
    for g in range(G):
        chunks = _score_chunks(g)
        nch = len(chunks)
        if stages == "full":
            zp = zpool.tile([128, nch], F32, tag="zp")
        else:
            zp = None
        for ci, (q0, w) in enumerate(chunks):
            pt = ps.tile([128, 512], F32, tag="ps")
            for dt_ in range(ET):
                lhs = (yt[dt_][:, g * 128:(g + 1) * 128] if stages == "full"
                       else xk[dt_][:, g * 128:(g + 1) * 128])
                nc.tensor.matmul(
                    pt[:, :w],
                    lhs,
                    xt[dt_][:, q0:q0 + w],
                    start=(dt_ == 0), stop=(dt_ == ET - 1),
                )
            if stages == "full":
                if ci == 0:
                    # masked (diagonal) region = first 256 valid columns
                    nc.vector.tensor_add(pt[:, :256], pt[:, :256], mb_sb[:])
                nc.scalar.activation(
                    wt_sb[:, WOFF[g] + (q0 - 256 * g): WOFF[g] + (q0 - 256 * g) + w],
                    pt[:, :w],
                    mybir.ActivationFunctionType.Exp,
                    scale=float(SCALE),
                    accum_out=zp[:, ci:ci + 1],
                )
        if stages == "full":
            vptile = vppool.tile([128, D], BF, tag="vp")
        else:
            vptile = None
        if stages == "full":
            z = zpool.tile([128, 1], F32, tag="z")
            nc.vector.tensor_reduce(z[:], zp[:], axis=mybir.AxisListType.X,
                                    op=mybir.AluOpType.add)
            rz = zpool.tile([128, 1], F32, tag="rz")
            nc.vector.reciprocal(rz[:], z[:])
            nc.vector.tensor_scalar_mul(vptile[:], vt[g][:], rz[:])
        vp[g] = vptile

        # AV for q-tiles 2g and 2g+1 (they need groups 0..g only)
        for qt in (2 * g, 2 * g + 1):
            split_tail = (qt >= NT - 2)  # stream the last tiles per 512-half
            stage = stpool.tile([128, D], BF, tag="st")
            for oc in range(2):
                apt = avps.tile([128, 512], F32, tag="av")
                for gg in range(g + 1):
                    if stages == "full":
                        lhs = wt_sb[:, WOFF[gg] + 128 * qt - 256 * gg:
                                       WOFF[gg] + 128 * qt - 256 * gg + 128]
                        rhs = vp[gg][:, oc * 512:(oc + 1) * 512]
                    else:
                        lhs = xk[gg][:, :128]
                        rhs = xk[gg][:, oc * 512:(oc + 1) * 512]
                    nc.tensor.matmul(apt[:], lhs, rhs,
                                     start=(gg == 0), stop=(gg == g))
                if stages == "full" or qt == NT - 1:
                    nc.vector.tensor_copy(stage[:, oc * 512:(oc + 1) * 512], apt[:])
                    if split_tail:
                        nc.sync.dma_start(
                            out_d[qt * 128:(qt + 1) * 128, oc * 512:(oc + 1) * 512],
                            stage[:, oc * 512:(oc + 1) * 512])
            if (stages == "full" or qt == NT - 1) and not split_tail:
                nc.sync.dma_start(out_d[qt * 128:(qt + 1) * 128, :], stage[:])


def build_program(with_loop=False, max_iters=64, stages="full", warmup=8,
                  hint_loop=True, staggered=False):
    """Build and compile the SPMD program. Returns the compiled Bacc."""
    nc = bacc.Bacc("TRN2", target_bir_lowering=False, debug=False, num_devices=8)
    xT_d = nc.dram_tensor("xT", [D, N], BF, kind="ExternalInput").ap()
    xkT_d = nc.dram_tensor("xkT", [D, D], BF, kind="ExternalInput").ap()
    a2_d = nc.dram_tensor("a2", [D, D], BF, kind="ExternalInput").ap()
    wvT_d = nc.dram_tensor("wvT", [D, D], BF, kind="ExternalInput").ap()
    mb_d = nc.dram_tensor("maskbias", [128, 256], F32, kind="ExternalInput").ap()
    out_d = nc.dram_tensor("out", [N, D], BF, kind="ExternalOutput").ap()
    if with_loop:
        n_d = nc.dram_tensor("niter", [1, 1], mybir.dt.int32,
                             kind="ExternalInput").ap()

    with tile.TileContext(nc) as tc:
        with ExitStack() as ctx:
            persist = ctx.enter_context(tc.tile_pool(name="persist", bufs=1))
            xpool = ctx.enter_context(tc.tile_pool(name="xT", bufs=ET))
            wpool = ctx.enter_context(tc.tile_pool(name="weights", bufs=2 * ET))
            ypool = ctx.enter_context(tc.tile_pool(name="yT", bufs=ET))
            vpool = ctx.enter_context(tc.tile_pool(name="v", bufs=G))
            vppool = ctx.enter_context(tc.tile_pool(name="vp", bufs=G))
            zpool = ctx.enter_context(tc.tile_pool(name="z", bufs=3 * G))
            stpool = ctx.enter_context(tc.tile_pool(name="stage", bufs=4))
            ps = ctx.enter_context(tc.tile_pool(name="ps", bufs=4, space="PSUM"))
            avps = ctx.enter_context(tc.tile_pool(name="avps", bufs=4, space="PSUM"))

            mb_sb = persist.tile([128, 256], F32, tag="mb")
            nc.scalar.dma_start(mb_sb[:], mb_d[:])
            wt_sb = persist.tile([128, WTOT], BF, tag="wt")
            wz_sb = persist.tile([128, 512], BF, tag="wz")
            if warmup:
                nc.vector.memset(wz_sb[:], 0.0)

            pools = (xpool, wpool, ypool, vpool, vppool, zpool, stpool, ps, avps)
            aps = (xT_d, xkT_d, a2_d, wvT_d, mb_sb, wt_sb, wz_sb, out_d)

            if with_loop:
                n_sb = persist.tile([1, 1], mybir.dt.int32, tag="niter")
                nc.sync.dma_start(n_sb[:], n_d[:])
                regs = []
                with tc.tile_critical():
                    for e, eng in nc.engines.items():
                        r = eng.alloc_register(f"niter_{e.name}")
                        eng.reg_load(r, n_sb[0:1, 0:1])
                        regs.append(r)
                n_val = nc.snap(bass.RegisterHandles(regs), min_val=0,
                                max_val=max_iters)
                loop_kw = {}
                if hint_loop:
                    loop_kw["hint_engines"] = tuple(nc.engines)
                if staggered:
                    loop_kw["staggered_reset"] = True
                with tc.For_i(0, n_val, 1, **loop_kw):
                    _emit_body(nc, tc, ctx, pools, aps, stages, warmup)
            else:
                _emit_body(nc, tc, ctx, pools, aps, stages, warmup)

    nc.compile()
    return nc


def prepare_in_maps(x, Wq, Wk, Wv, niter=None):
    """Host-side sharding: per-core input maps (8 cores)."""
    x = np.asarray(x, dtype=np.float32)
    A2 = (np.asarray(Wk, np.float32).T @ np.asarray(Wq, np.float32))  # [e, d]
    a2_bf = A2.astype(ml_dtypes.bfloat16)
    wvT_bf = np.asarray(Wv, np.float32).T.astype(ml_dtypes.bfloat16)  # [e, o]
    mb = []
    for h in range(2):
        i = np.arange(128)[:, None]
        j = np.arange(256)[None, :]
        mb.append(np.where(j >= 2 * i + h, 0.0, NEGBIG).astype(np.float32))
    in_maps = []
    for c in range(8):
        b, h = c // 2, c % 2
        xTb = x[b].T.astype(ml_dtypes.bfloat16)
        m = {
            "xT": xTb,
            "xkT": np.ascontiguousarray(xTb[:, h::2]),
            "a2": a2_bf,
            "wvT": wvT_bf,
            "maskbias": mb[h],
        }
        if niter is not None:
            m["niter"] = np.array([[niter]], dtype=np.int32)
        in_maps.append(m)
    return in_maps


_CACHE = {}


def kernel(x, Wq, Wk, Wv):
    if "nc" not in _CACHE:
        _CACHE["nc"] = build_program(with_loop=False)
    nc = _CACHE["nc"]
    in_maps = prepare_in_maps(x, Wq, Wk, Wv)
    res = run_bass_kernel_spmd(nc, in_maps, list(range(8)), trace=False)
    out = np.empty((B, N, D), np.float32)
    for b in range(B):
        out[b] = (res.results[2 * b]["out"].astype(np.float32)
                  + res.results[2 * b + 1]["out"].astype(np.float32))
    return out
